# revision 5
# baseline (speedup 1.0000x reference)
"""AttentionBlock (GroupNorm + single-head-group attention + out-proj + residual)
for Trainium2, data-parallel over batch across 8 NeuronCores.

Reference computation (per batch element, fp32 reference):
  hn  = GroupNorm32(x)                      # x: (C=512, L=1024)
  q   = Wq @ hn + bq ; kv = Wkv @ hn + bkv ; k, v = split(kv)
  per head h (8 heads, dh=64):
    dots = (q*s)^T (k*s), s = dh^-0.5       # scale applied to both q and k
    attn = softmax(dots, axis=s)
    out  = attn @ v^T  -> (dh, L)
  y = Wo @ out + bo + x

Layout strategy (avoids all large transposes):
  - channels on partitions for x/hn/q/k; v computed TRANSPOSED (L on
    partitions) directly from the projection (lhsT=hn);
  - scores computed transposed: dotsT[s,t] = kh^T qh (lhsT=kh, rhs=qh);
  - head pairs (2h, 2h+1) live at partition bases 0/64 of one chunk; k is
    zero-padded to full K=128 so dots matmuls keep the PE activity monitor
    fed (the steady state is ACT-bound, so the extra PE cycles are free);
  - softmax denominator via ones-columns appended to the vT weight tile
    (rows 64:128 of the AV psum = sum_s exp); normalization applied at AV
    evacuation (copy + fast-reciprocal + multiply on DVE);
  - exp() numerically safe without max-subtraction: |dots| < 0.5 here;
  - ONE activation table set for the whole kernel (natural_log_exp):
    rstd = exp(-0.5*ln(var+eps)), preloaded via a dummy Ln at t=0, so the
    ACT engine spends its time exclusively on the 64 softmax exp tiles;
  - GroupNorm sum-of-squares on DVE (affine_mul_reduce), not ACT;
  - v bias folded into the output projection: bo2 = Wo @ bv + bo computed
    on-device with N=1 matmuls (softmax rows sum to 1), killing the
    per-pair av-bias pass and decoupling out-proj from it;
  - out-proj is emitted th-major and accumulates per cj so its matmuls can
    overlap the tail of the attention pipeline;
  - matmul operands in bf16 (fp32 matmul costs 2x on the PE and disables
    fast weight load); psum accumulation, groupnorm statistics, softmax
    normalization and the residual add stay fp32.
"""

import numpy as np
import ml_dtypes

import concourse.bass as bass
import concourse.mybir as mybir
import concourse.tile as tile
from concourse import bacc, bass_utils
from concourse.bass import ts

F32 = mybir.dt.float32
BF16 = mybir.dt.bfloat16
AF = mybir.ActivationFunctionType
OP = mybir.AluOpType

B = 8
C = 512
HW = 32
L = HW * HW  # 1024
H = 8
DH = C // H  # 64
G = 32
GS = C // G  # 16
EPS = 1e-5
SCALE2 = float(DH) ** -1.0  # (dh^-0.5) applied to BOTH q and k -> 1/dh on dots
P = 128
CCH = C // P  # 4 channel chunks
LCH = L // P  # 8 L chunks
NCORES = 8
VW = H * P  # 1024: v^T tiles hold [64 v cols | 64 ones cols] per head

# params tile column blocks (each CCH wide): gamma, beta, bq, bk, bv, bo
PG, PB, PQ, PK, PV, PO = (i * CCH for i in range(6))
NPARAM = 6 * CCH


def _body(tc, tensors):
    nc = tc.nc
    from contextlib import ExitStack

    ctx = ExitStack()
    with ctx:
        persist = ctx.enter_context(tc.tile_pool(name="persist", bufs=1))
        work = ctx.enter_context(tc.tile_pool(name="work", bufs=4))
        expp = ctx.enter_context(tc.tile_pool(name="expp", bufs=32))
        outp = ctx.enter_context(tc.tile_pool(name="outp", bufs=3))
        ps_dots = ctx.enter_context(tc.tile_pool(name="ps_dots", bufs=2, space="PSUM"))
        ps_att = ctx.enter_context(tc.tile_pool(name="ps_att", bufs=2, space="PSUM"))
        ps_gen = ctx.enter_context(tc.tile_pool(name="ps_gen", bufs=2, space="PSUM"))

        x_d = tensors["x"].ap()
        xb_d = tensors["xb"].ap()
        params_d = tensors["params"].ap()
        wqT_d = tensors["wqT"].ap()
        wkvT_d = tensors["wkvT"].ap()
        woT_d = tensors["woT"].ap()
        ind_d = tensors["ind"].ap()
        indT_d = tensors["indT"].ap()
        out_d = tensors["out"].ap()

        # ---- ACT table preload: dummy Ln so natural_log_exp set loads at t=0
        warm = persist.tile([1, 1], F32, tag="warm")
        nc.vector.memset(warm, 1.0)
        nc.scalar.activation(warm, warm, AF.Ln)

        # ---------------- input DMAs ----------------
        # Issue cost is ~0.7-1.5us of ENGINE time per dma_start, so they are
        # spread over sync+gpsimd only (ACT is the kernel bottleneck, DVE is
        # number two -- neither issues any DMA).
        xbt = []
        xb3 = xb_d.rearrange("(cc p) l -> cc p l", p=P)
        x_engines = [nc.sync, nc.gpsimd, nc.sync, nc.gpsimd]
        for cj in range(CCH):
            t = persist.tile([P, L], BF16, tag=f"xb{cj}")
            x_engines[cj].dma_start(t, xb3[cj])
            xbt.append(t)

        params_t = persist.tile([P, NPARAM], F32, tag="params")
        nc.sync.dma_start(params_t, params_d)
        ind_t = persist.tile([P, CCH, G], F32, tag="ind")
        nc.gpsimd.dma_start(ind_t, ind_d.rearrange("(cc p) g -> p cc g", p=P))
        indT_t = persist.tile([G, C], F32, tag="indT")
        nc.gpsimd.dma_start(indT_t, indT_d)

        # weights, split per channel-chunk so the first projection matmuls
        # can start as soon as their chunk lands; k-half of wkv before v-half
        wq_t = persist.tile([P, CCH, C], BF16, tag="wq")
        wq4 = wqT_d.rearrange("p (cc o) -> p cc o", cc=CCH)
        wkv_t = persist.tile([P, CCH, 2 * C], BF16, tag="wkv")
        wkv4 = wkvT_d.rearrange("p (cc o) -> p cc o", cc=CCH)
        w_engines = [nc.sync, nc.gpsimd]
        for cj in range(CCH):
            w_engines[cj % 2].dma_start(wq_t[:, cj, :], wq4[:, cj, :])
            w_engines[(cj + 1) % 2].dma_start(
                wkv_t[:, cj, 0:C], wkv4[:, cj, 0:C]
            )
        for cj in range(CCH):
            w_engines[cj % 2].dma_start(
                wkv_t[:, cj, C : 2 * C], wkv4[:, cj, C : 2 * C]
            )
        wo_t = persist.tile([P, CCH, C], BF16, tag="wo")
        wo4 = woT_d.rearrange("p (cc o) -> p cc o", cc=CCH)
        for cj in range(CCH):
            w_engines[cj % 2].dma_start(wo_t[:, cj, :], wo4[:, cj, :])

        xt = []  # fp32 x for the residual, loads in the background
        x3 = x_d.rearrange("(cc p) l -> cc p l", p=P)
        for cj in range(CCH):
            t = persist.tile([P, L], F32, tag=f"x{cj}")
            x_engines[cj].dma_start(t, x3[cj])
            xt.append(t)

        eps_t = persist.tile([G, 1], F32, tag="eps")
        nc.vector.memset(eps_t, EPS)

        # ---------------- GroupNorm ----------------
        # per-channel [sum, sumsq] on DVE only -> group-reduce via indicator
        stats = work.tile([P, CCH, 2], F32, tag="stats")
        for cj in range(CCH):
            sq = work.tile([P, L], BF16, tag="sq")
            nc.vector.affine_mul_reduce(
                sq, stats[:, cj, 1:2], xbt[cj], xbt[cj], 1.0, 0.0
            )
            nc.vector.reduce_sum(stats[:, cj, 0:1], xbt[cj], axis=mybir.AxisListType.X)

        ps_stats = ps_gen.tile([G, 2], F32, tag="ps")
        for cj in range(CCH):
            nc.tensor.matmul(
                ps_stats,
                ind_t[:, cj, :],
                stats[:, cj, :],
                start=(cj == 0),
                stop=(cj == CCH - 1),
            )

        # mv = [mean, rstd] per group (G partitions); rstd via exp(-.5 ln(v+eps))
        mv = work.tile([G, 2], F32, tag="mv")
        inv_n = 1.0 / (GS * L)
        nc.scalar.mul(mv[:, 0:1], ps_stats[:, 0:1], inv_n)  # mean
        nc.scalar.mul(mv[:, 1:2], ps_stats[:, 1:2], inv_n)  # E[x^2]
        musq = work.tile([G, 1], F32, tag="musq")
        nc.vector.tensor_mul(musq, mv[:, 0:1], mv[:, 0:1])
        nc.vector.tensor_tensor(mv[:, 1:2], mv[:, 1:2], musq, OP.subtract)  # var
        nc.scalar.activation(mv[:, 1:2], mv[:, 1:2], AF.Ln, bias=eps_t)
        nc.scalar.activation(mv[:, 1:2], mv[:, 1:2], AF.Exp, scale=-0.5)  # rstd

        # broadcast group stats back to channels: (G,2) -> (128,2) per chunk
        hn = []
        for cj in range(CCH):
            ps_bcst = ps_gen.tile([P, 2], F32, tag="ps")
            nc.tensor.matmul(ps_bcst, indT_t[:, ts(cj, P)], mv, start=True, stop=True)
            mc = work.tile([P, 2], F32, tag="mc")
            nc.vector.tensor_copy(mc, ps_bcst)
            a = work.tile([P, 1], F32, tag="a_sc")
            b = work.tile([P, 1], F32, tag="b_sc")
            # a = rstd*gamma ; b = beta - mean*a
            nc.vector.tensor_mul(a, mc[:, 1:2], params_t[:, PG + cj : PG + cj + 1])
            nc.vector.tensor_mul(b, mc[:, 0:1], a)
            nc.vector.tensor_tensor(
                b, params_t[:, PB + cj : PB + cj + 1], b, OP.subtract
            )
            t = persist.tile([P, L], BF16, tag=f"hn{cj}")
            nc.vector.tensor_scalar(
                t, xbt[cj], scalar1=a, scalar2=b, op0=OP.mult, op1=OP.add
            )
            hn.append(t)

        # ---------------- projections ----------------
        # q (channels on partitions), pre-scaled by 1/dh; k (channels on
        # partitions, zero-padded per head so dots run full-K);
        # vT (L on partitions) with per-head ones-columns appended.
        q_t = [persist.tile([P, L], BF16, tag=f"q{oj}", name=f"q{oj}") for oj in range(CCH)]
        kp_t = [persist.tile([P, L], BF16, tag=f"kp{h}", name=f"kp{h}") for h in range(H)]
        for h in range(H):
            base = DH * (h % 2)
            nc.vector.memset(kp_t[h][DH - base : P - base, :], 0.0)
        vT = [persist.tile([P, VW], BF16, tag=f"vT{lj}", name=f"vT{lj}") for lj in range(LCH)]

        bo2_t = persist.tile([P, CCH], F32, tag="bo2")
        bv16_t = persist.tile([P, CCH], BF16, tag="bv16")
        nc.vector.tensor_copy(bv16_t, params_t[:, PV : PV + CCH])

        def emit_qk(oj):
            for th in range(2):
                ps_q = ps_gen.tile([P, 512], F32, tag="ps", name="ps_q")
                for cj in range(CCH):
                    nc.tensor.matmul(
                        ps_q,
                        wq_t[:, cj, ts(oj, P)],
                        hn[cj][:, ts(th, 512)],
                        start=(cj == 0),
                        stop=(cj == CCH - 1),
                    )
                # q = (psum + bq) * (1/dh)
                nc.vector.tensor_scalar(
                    q_t[oj][:, ts(th, 512)],
                    ps_q,
                    scalar1=params_t[:, PQ + oj : PQ + oj + 1],
                    scalar2=SCALE2,
                    op0=OP.add,
                    op1=OP.mult,
                )
                ps_k = ps_gen.tile([P, 512], F32, tag="ps", name="ps_k")
                for cj in range(CCH):
                    nc.tensor.matmul(
                        ps_k,
                        wkv_t[:, cj, ts(oj, P)],
                        hn[cj][:, ts(th, 512)],
                        start=(cj == 0),
                        stop=(cj == CCH - 1),
                    )
                nc.vector.tensor_scalar(
                    kp_t[2 * oj][0:DH, ts(th, 512)],
                    ps_k[0:DH, :],
                    scalar1=params_t[0:DH, PK + oj : PK + oj + 1],
                    scalar2=None,
                    op0=OP.add,
                )
                nc.vector.tensor_scalar(
                    kp_t[2 * oj + 1][DH:P, ts(th, 512)],
                    ps_k[DH:P, :],
                    scalar1=params_t[DH:P, PK + oj : PK + oj + 1],
                    scalar2=None,
                    op0=OP.add,
                )

        def emit_vt(lj):
            # vT: out[l, i] = sum_c hn[c, l] * Wv^T[c, i]  (lhsT = hn chunks)
            v3 = vT[lj].rearrange("p (h w) -> p h w", w=P)
            nc.vector.memset(v3[:, :, DH:P], 1.0)
            ps_v = ps_gen.tile([P, 512], F32, tag="ps", name="ps_v")
            for cj in range(CCH):
                nc.tensor.matmul(
                    ps_v,
                    hn[cj][:, ts(lj, P)],
                    wkv_t[:, cj, C : 2 * C],
                    start=(cj == 0),
                    stop=(cj == CCH - 1),
                )
            # v bias is folded into bo2 (softmax rows sum to one).
            # single strided copy: psum (p,(h d)) -> vT (p,h,0:DH)
            nc.vector.tensor_copy(
                v3[:, :, 0:DH], ps_v.rearrange("p (h d) -> p h d", d=DH)
            )

        def emit_bo2(oj):
            # bo2 = Wo @ bv + bo (v bias folded through the out projection)
            ps_b = ps_gen.tile([P, 1], F32, tag="ps", name="ps_b")
            for cj in range(CCH):
                nc.tensor.matmul(
                    ps_b,
                    wo_t[:, cj, ts(oj, P)],
                    bv16_t[:, cj : cj + 1],
                    start=(cj == 0),
                    stop=(cj == CCH - 1),
                )
            nc.vector.tensor_tensor(
                bo2_t[:, oj : oj + 1], ps_b, params_t[:, PO + oj : PO + oj + 1], OP.add
            )

        emit_qk(0)
        # remaining projections + bo2 are dripped into the pair pipeline as
        # PE filler while the ACT exp pipeline paces the dots psums
        fillers = [lambda oj=oj: emit_qk(oj) for oj in range(1, CCH)]
        fillers += [lambda lj=lj: emit_vt(lj) for lj in range(LCH)]
        fillers += [lambda oj=oj: emit_bo2(oj) for oj in range(CCH)]

        out3 = out_d.rearrange("(cc p) l -> cc p l", p=P)

        # ---------------- attention, head-pair pipelined ----------------
        av_t = [persist.tile([P, L], BF16, tag=f"av{oj}", name=f"av{oj}") for oj in range(CCH)]
        exp_tiles: dict = {}

        def emit_av_evac(h, th, ps_o):
            oj, base = h // 2, DH * (h % 2)
            # psum rows 64:128 hold sum_s exp (replicated via the
            # ones columns of vT). Copy to p0, fast-reciprocal
            # (same-partition custom op), multiply rows 0:64.
            se = work.tile([DH, 512], F32, tag="se")
            nc.vector.tensor_copy(se, ps_o[DH:P, :])
            rec = work.tile([DH, 512], F32, tag="rec")
            nc.vector.reciprocal_approx_fast(rec, se)
            nc.vector.tensor_tensor(
                av_t[oj][base : base + DH, ts(th, 512)],
                ps_o[:DH, :],
                rec,
                OP.mult,
            )

        def av_thunks(hp):
            # AV matmuls for pair hp, group-major (h, th), sj ascending inside
            # a group; each group accumulates into one 1-bank psum then evacs.
            thunks = []
            for h in (2 * hp, 2 * hp + 1):
                for th in range(2):
                    state = {}

                    def mk(h=h, th=th, state=state):
                        def first():
                            state["ps"] = ps_att.tile(
                                [P, 512], F32, tag="ps", name="ps_av"
                            )
                            _mm(0)

                        def _mm(sj):
                            nc.tensor.matmul(
                                state["ps"],
                                vT[sj][:, ts(h, P)],
                                exp_tiles[(h, sj)][:, ts(th, 512)],
                                start=(sj == 0),
                                stop=(sj == LCH - 1),
                            )

                        out = [first]
                        out += [lambda sj=sj: _mm(sj) for sj in range(1, LCH)]
                        out += [lambda: emit_av_evac(h, th, state["ps"])]
                        return out

                    thunks += mk()
            return thunks

        def emit_pair(hp, av_q, fill_q, n_fill):
            # dots+exp for pair hp (if not None), interleaved with AV work of
            # the previous pair and filler projections, per (h, sj) slot.
            slots = [(h, sj) for sj in range(LCH) for h in (2 * hp, 2 * hp + 1)] if hp is not None else [(None, None)] * 16
            for si, (h, sj) in enumerate(slots):
                for _ in range(n_fill):
                    if fill_q:
                        fill_q.pop(0)()
                nav = (len(av_q) + (16 - si) - 1) // (16 - si) if av_q else 0
                for _ in range(nav):
                    if av_q:
                        av_q.pop(0)()
                if h is None:
                    continue
                oj = hp
                ps_d = ps_dots.tile([P, L], F32, tag="ps", name="ps_d")
                for th in range(2):
                    nc.tensor.matmul(
                        ps_d[:, ts(th, 512)],
                        kp_t[h][:, ts(sj, P)],
                        q_t[oj][:, ts(th, 512)],
                        start=True,
                        stop=True,
                    )
                e = expp.tile([P, L], BF16, tag="exp", name="exp_e")
                nc.scalar.activation(e, ps_d, AF.Exp)
                exp_tiles[(h, sj)] = e

        # pair 0: no AV yet, drip fillers (1 per slot)
        emit_pair(0, [], fillers, 1)
        for hp in range(1, CCH):
            emit_pair(hp, av_thunks(hp - 1), fillers, 1)
        emit_pair(None, av_thunks(CCH - 1), fillers, 1)

        # ---------------- output projection + residual ----------------
        out_engines = [nc.sync, nc.gpsimd, nc.sync, nc.gpsimd]
        for th in range(2):
            for oj in range(CCH):
                ps_f = ps_gen.tile([P, 512], F32, tag="ps", name="ps_f")
                for cj in range(CCH):
                    nc.tensor.matmul(
                        ps_f,
                        wo_t[:, cj, ts(oj, P)],
                        av_t[cj][:, ts(th, 512)],
                        start=(cj == 0),
                        stop=(cj == CCH - 1),
                    )
                ot = outp.tile([P, 512], F32, tag="ot")
                # ot = (psum + bo2) + x  in one DVE pass
                nc.vector.affine_then_add(
                    ot,
                    ps_f,
                    xt[oj][:, ts(th, 512)],
                    scale=1.0,
                    bias=bo2_t[:, oj : oj + 1],
                )
                out_engines[(2 * oj + th) % 4].dma_start(out3[oj][:, ts(th, 512)], ot)


_CACHE = {}


def _build():
    if "nc" in _CACHE:
        return _CACHE["nc"]
    nc = bacc.Bacc("TRN2", target_bir_lowering=False, debug=False, num_devices=NCORES)
    tensors = {}
    specs = [
        ("x", (C, L), F32),
        ("xb", (C, L), BF16),
        ("params", (P, NPARAM), F32),
        ("wqT", (P, CCH * C), BF16),
        ("wkvT", (P, CCH * 2 * C), BF16),
        ("woT", (P, CCH * C), BF16),
        ("ind", (C, G), F32),
        ("indT", (G, C), F32),
    ]
    for name, shape, dt in specs:
        tensors[name] = nc.dram_tensor(name, shape, dt, kind="ExternalInput")
    tensors["out"] = nc.dram_tensor("out", (C, L), F32, kind="ExternalOutput")
    with tile.TileContext(nc) as tc:
        _body(tc, tensors)
    nc.compile()
    _CACHE["nc"] = nc
    return nc


def _in_maps(x, gamma, beta, Wq, bq, Wkv, bkv, Wo, bo):
    f32 = lambda a: np.ascontiguousarray(np.asarray(a, dtype=np.float32))

    def shuf(wT):
        # (c, o) -> (p, cc*o), c = cc*128 + p: one contiguous row per partition
        c, o = wT.shape
        return wT.reshape(c // P, P, o).transpose(1, 0, 2).reshape(P, -1)

    bf16 = lambda a: np.ascontiguousarray(
        np.asarray(a, dtype=np.float32).astype(ml_dtypes.bfloat16)
    )
    xr = f32(x).reshape(B, C, L)
    ind = np.zeros((C, G), np.float32)
    ind[np.arange(C), np.arange(C) // GS] = 1.0

    def cols(v):
        # (C,) -> (P, CCH) where column cj <-> channels cj*128..+128
        return np.asarray(v, np.float32).reshape(CCH, P).T

    bkv_a = np.asarray(bkv, np.float32)
    params = np.concatenate(
        [cols(gamma), cols(beta), cols(bq), cols(bkv_a[:C]), cols(bkv_a[C:]), cols(bo)],
        axis=1,
    )
    shared = {
        "params": np.ascontiguousarray(params),
        "wqT": bf16(shuf(np.asarray(Wq, np.float32).T)),
        "wkvT": bf16(shuf(np.asarray(Wkv, np.float32).T)),
        "woT": bf16(shuf(np.asarray(Wo, np.float32).T)),
        "ind": ind,
        "indT": f32(ind.T),
    }
    return [
        dict(
            shared,
            x=np.ascontiguousarray(xr[i]),
            xb=np.ascontiguousarray(xr[i].astype(ml_dtypes.bfloat16)),
        )
        for i in range(B)
    ]


def kernel(x, gamma, beta, Wq, bq, Wkv, bkv, Wo, bo):
    nc = _build()
    in_maps = _in_maps(x, gamma, beta, Wq, bq, Wkv, bkv, Wo, bo)
    res = bass_utils.run_bass_kernel_spmd(nc, in_maps, core_ids=list(range(NCORES)))
    out = np.stack([res.results[i]["out"] for i in range(B)], axis=0)
    return out.reshape(B, C, HW, HW).astype(np.float32)
